# revision 27
# baseline (speedup 1.0000x reference)
"""Trainium2 Bass kernel for nn_Baseline_GNN (gnn_message_passing).

Data-parallel over batch across 8 NeuronCores. Per-core pipeline (fp16
activations, fp32 PSUM/stats):

  per layer l (3x):
    AGG:   v.T = (maskT + eps*I)_s @ h_s per sample via PE (mask stationary
           chunks are h_row slices, moving operand is maskT), output
           feature-major in PSUM -> SBUF fp16.
    Z1:    z1.T = W1.T-chunks @ v.T   (PE), PSUM->SBUF fp16 copy (ACT) +
           bn_stats (DVE) per tile.
    BN1:   cross-core AllReduce of (sum, sumsq) per feature -> scale/shift.
    ELU:   u = min(exp(n),1)-1+relu(n), n = s*z+t   (ACT exp/relu + DVE).
    Z2/BN2/ELU -> w ; BN3 (stats via bn_stats on w) /ELU -> h'.
    h'.T -> h_row via per-sample DMA transposes (fp16).
  final: xm = row-sum over roi (DVE reduce; 1/200 folded into Wm1),
         AllGather xm, replicated tiny MLP with local BN stats, y (256,2).

b1/b2/bm1 are mathematically dropped (train-mode BN subtracts the mean, so
per-feature constant biases cancel exactly).
"""
import numpy as np
import ml_dtypes

import concourse.bass as bass
import concourse.mybir as mybir
import concourse.tile as tile
import concourse.bacc as bacc
from concourse.bass_utils import run_bass_kernel_spmd

F32 = mybir.dt.float32
F16 = mybir.dt.float16
AF = mybir.ActivationFunctionType
A = mybir.AluOpType

B, ROI, T, L = 256, 200, 512, 3
NCORES = 8
S = B // NCORES            # samples per core
RPC = S * ROI              # rows per core (6400)
FC = T // 128              # feature chunks (4)
NBLK = (RPC + 511) // 512  # 13 dense row-blocks (12x512 + 256)
PADC = RPC + 256           # padded free dim for transpose source buffer
NG = float(B * ROI)        # global BN row count
NLOC = float(RPC)
BN_EPS = 1e-5


def _blk(rb):
    off = rb * 512
    return off, min(512, RPC - off)


import os
LAYER_REP = int(os.environ.get("K_LAYER_REP", "1"))
SKIP_AR = os.environ.get("K_SKIP_AR", "") == "1"
NO_APPLY = os.environ.get("K_NO_APPLY", "") == "1"
NO_STATS = os.environ.get("K_NO_STATS", "") == "1"
NO_TRANS = os.environ.get("K_NO_TRANS", "") == "1"
NO_DENSE = os.environ.get("K_NO_DENSE", "") == "1"
NO_AGG = os.environ.get("K_NO_AGG", "") == "1"


def build_nc():
    nc = bacc.Bacc("TRN2", target_bir_lowering=False, debug=False,
                   num_devices=NCORES)

    xr = nc.dram_tensor("xr", [S, ROI, T], F16, kind="ExternalInput")
    mk = nc.dram_tensor("mk", [L, S, ROI, ROI], F16, kind="ExternalInput")
    w12 = nc.dram_tensor("w12", [L, 2, 128, FC, T], F16, kind="ExternalInput")
    bnp = nc.dram_tensor("bnp", [L, 6, 128, FC], F32, kind="ExternalInput")
    wm1 = nc.dram_tensor("wm1", [128, FC, 256], F16, kind="ExternalInput")
    wm2 = nc.dram_tensor("wm2", [128, 2, 2], F16, kind="ExternalInput")
    fbn = nc.dram_tensor("fbn", [128, 5], F32, kind="ExternalInput")
    idm = nc.dram_tensor("idm", [128, 128], F16, kind="ExternalInput")
    y = nc.dram_tensor("y", [B, 2], F32, kind="ExternalOutput")

    with tile.TileContext(nc) as tc:
        with (
            tc.tile_pool(name="big", bufs=1) as big,
            tc.tile_pool(name="wts", bufs=1) as wts,
            tc.tile_pool(name="mskp", bufs=3) as mskp,
            tc.tile_pool(name="esc", bufs=3) as esc,
            tc.tile_pool(name="stp", bufs=2) as stp,
            tc.tile_pool(name="stt", bufs=4) as stt,
            tc.tile_pool(name="dram", bufs=1, space="DRAM") as dram,
            tc.tile_pool(name="aps", bufs=2, space="PSUM") as aps,
            tc.tile_pool(name="dps", bufs=4, space="PSUM") as dps,
        ):
            # --- persistent big activation buffers ---
            bufA = big.tile([128, FC, RPC], F16)          # vT / z2T
            bufB = big.tile([128, FC, RPC], F16)          # z1T / wT
            bufC = big.tile([128, FC, PADC], F16)         # uT / h'T (padded)
            hrow = big.tile([128, 6, 2, T], F16)          # slots x (a,b) halves
            idt = big.tile([128, 128], F16)
            nc.sync.dma_start(idt[:], idm.ap())

            nc.vector.memset(bufC[:, :, RPC:], 0.0)
            bnpt = big.tile([128, L, 6, FC], F32)
            nc.sync.dma_start(bnpt[:], bnp.ap().rearrange("l k p c -> p l k c"))
            fbnt = big.tile([128, 5], F32)
            nc.sync.dma_start(fbnt[:], fbn.ap())
            wm1t = big.tile([128, FC, 256], F16)
            nc.sync.dma_start(wm1t[:], wm1.ap())
            wm2t = big.tile([128, 2, 2], F16)
            nc.sync.dma_start(wm2t[:], wm2.ap())

            def load_weights(l):
                wt = wts.tile([128, 2, FC, T], F16, name=f"wt{np.random.randint(1<<30)}", tag="wt")
                nc.sync.dma_start(wt[:], w12.ap()[l].rearrange("w p c t -> p w c t"))
                return wt

            def bn_sync(st6, nchunks, l, gk, bek, tag, mv=None):
                """Aggregate bn_stats chunks, AllReduce, return (s,t) (128,FC)."""
                if mv is None:
                    ag = stt.tile([128, FC, 2], F32, name=f"ag{tag}", tag="ag")
                    if NO_STATS:
                        nc.vector.memset(st6[:], 1.0)
                    for fo in range(FC):
                        nc.vector.bn_aggr(
                            ag[:, fo], st6[:, fo, :nchunks].rearrange("p c s -> p (c s)"))
                    mean = ag[:, :, 0:1].rearrange("p c o -> p (c o)")
                    var = ag[:, :, 1:2].rearrange("p c o -> p (c o)")
                else:
                    mean, var = mv
                pay = stt.tile([128, 2 * FC], F32, name=f"pay{tag}", tag="pay")
                msq = stt.tile([128, FC], F32, name=f"msq{tag}", tag="msq")
                nc.vector.tensor_tensor(msq[:], mean, mean, A.mult)
                nc.vector.tensor_tensor(pay[:, FC:], msq[:], var, A.add)
                nc.vector.tensor_scalar(pay[:, FC:], pay[:, FC:], NLOC, 0.0,
                                        A.mult, A.add)
                nc.vector.tensor_scalar(pay[:, :FC], mean, NLOC, 0.0,
                                        A.mult, A.add)
                bin_ = dram.tile([128, 2 * FC], F32, name=f"bin{tag}")
                bout = dram.tile([128, 2 * FC], F32, name=f"bout{tag}",
                                 addr_space="Shared")
                nc.sync.dma_start(bin_[:], pay[:])
                gp = stt.tile([128, 2 * FC], F32, name=f"gp{tag}", tag="gp")
                if SKIP_AR:
                    nc.sync.dma_start(gp[:], bin_[:])
                else:
                    nc.gpsimd.collective_compute(
                        "AllReduce", A.add, ins=[bin_[:].opt()], outs=[bout[:].opt()],
                        replica_groups=[list(range(NCORES))])
                    nc.sync.dma_start(gp[:], bout[:])
                mg = stt.tile([128, FC], F32, name=f"mg{tag}", tag="mg")
                vg = stt.tile([128, FC], F32, name=f"vg{tag}", tag="vg")
                nc.vector.tensor_scalar(mg[:], gp[:, :FC], 1.0 / NG, 0.0,
                                        A.mult, A.add)
                nc.vector.tensor_scalar(vg[:], gp[:, FC:], 1.0 / NG, 0.0,
                                        A.mult, A.add)
                nc.vector.tensor_tensor(msq[:], mg[:], mg[:], A.mult)
                nc.vector.tensor_tensor(vg[:], vg[:], msq[:], A.subtract)
                # rstd = exp(-0.5*ln(var+eps)) ; ln & exp share one ACT table set
                nc.vector.tensor_scalar(vg[:], vg[:], 1.0, BN_EPS, A.mult, A.add)
                nc.scalar.activation(vg[:], vg[:], AF.Ln, bias=0.0, scale=1.0)
                nc.scalar.activation(vg[:], vg[:], AF.Exp, bias=0.0, scale=-0.5)
                st_s = stt.tile([128, FC], F32, name=f"s{tag}", tag="s")
                st_t = stt.tile([128, FC], F32, name=f"t{tag}", tag="t")
                nc.vector.tensor_tensor(st_s[:], vg[:], bnpt[:, l, gk], A.mult)
                nc.vector.tensor_tensor(msq[:], mg[:], st_s[:], A.mult)
                nc.vector.tensor_tensor(st_t[:], bnpt[:, l, bek], msq[:],
                                        A.subtract)
                return st_s, st_t

            def apply_elu(zT, uT, st_s, st_t, tag):
                """u = ELU(n) = max(n, min(exp(n),1)-1), n = s*z+t, blockwise."""
                NAB = (RPC + 1023) // 1024
                if NO_APPLY:
                    for fc in range(FC):
                        nc.vector.tensor_copy(uT[:, fc, :RPC], zT[:, fc, :RPC])
                    return
                for rb in range(NAB):
                    off = rb * 1024
                    n = min(1024, RPC - off)
                    for fc in range(FC):
                        src = zT[:, fc, off:off + n]
                        sA = st_s[:, fc:fc + 1]
                        tA = st_t[:, fc:fc + 1]
                        e = esc.tile([128, 1024], F16, name=f"e{tag}_{rb}_{fc}",
                                     tag="eblk")
                        r = esc.tile([128, 1024], F16, name=f"r{tag}_{rb}_{fc}",
                                     tag="rblk")
                        nc.scalar.activation(e[:, :n], src, AF.Exp,
                                             bias=tA, scale=sA)
                        nc.vector.tensor_scalar(r[:, :n], src, sA, tA,
                                                A.mult, A.add)
                        nc.vector.tensor_scalar(e[:, :n], e[:, :n], 1.0, -1.0,
                                                A.min, A.add)
                        nc.vector.tensor_tensor(uT[:, fc, off:off + n],
                                                r[:, :n], e[:, :n], A.max)

            def dense(wt, wi, srcT, dstT, st6):
                """dstT = (W.T @ srcT) per chunk; PSUM->SBUF + bn_stats."""
                for rb in range(NBLK):
                    off, n = _blk(rb)
                    for fo in range(FC):
                        ps = dps.tile([128, 512], F32,
                                      name=f"dps{rb}_{fo}", tag="dpst")
                        for fi in range(FC) if not NO_DENSE else [0]:
                            nc.tensor.matmul(
                                ps[:, :n],
                                wt[:, wi, fi, fo * 128:(fo + 1) * 128],
                                srcT[:, fi, off:off + n],
                                start=(fi == 0),
                                stop=True if NO_DENSE else (fi == FC - 1))
                        nc.scalar.activation(dstT[:, fo, off:off + n],
                                             ps[:, :n], AF.Copy)
                        if not NO_STATS:
                            nc.vector.bn_stats(st6[:, fo, rb],
                                               dstT[:, fo, off:off + n])

            def agg(l, first):
                """Per-sample aggregation: vT (bufA) = (maskT_eps) @ h."""
                if NO_AGG:
                    nc.vector.memset(bufA[:, :, 0:4], 0.0)
                    return
                mag = mbg = None
                for s in range(S):
                    slot = s % 6
                    mslot = s % 4
                    if s % 4 == 0:
                        mag = mskp.tile([128, 4, ROI], F16,
                                        name=f"ma{l}_{s}_{id(wt)}", tag="ma")
                        mbg = mskp.tile([128, 4, ROI], F16,
                                        name=f"mb{l}_{s}_{id(wt)}", tag="mb")
                        nc.sync.dma_start(
                            mag[:], mk.ap()[l, s:s + 4, 0:128, :].rearrange(
                                "s j i -> j s i"))
                        nc.sync.dma_start(
                            mbg[:72], mk.ap()[l, s:s + 4, 128:200, :].rearrange(
                                "s j i -> j s i"))
                    ma = mag[:, mslot]
                    mb = mbg[:, mslot]
                    if first:
                        nc.sync.dma_start(hrow[:, slot, 0, :], xr.ap()[s, 0:128, :])
                        nc.sync.dma_start(hrow[0:72, slot, 1, :], xr.ap()[s, 128:200, :])
                    elif not NO_TRANS:
                        for fcx in range(FC):
                            c0 = s * ROI
                            tp = dps.tile([128, 512], F16,
                                          name=f"tp{l}_{s}_{fcx}", tag="dpst")
                            nc.tensor.matmul(tp[:, 0:128],
                                             bufC[:, fcx, c0:c0 + 128], idt[:],
                                             is_transpose=True, start=True,
                                             stop=False)
                            nc.tensor.matmul(tp[:, 128:256],
                                             bufC[:, fcx, c0 + 128:c0 + 256],
                                             idt[:], is_transpose=True,
                                             start=False, stop=True,
                                             skip_group_check=True)
                            dst = hrow[:, slot, :, fcx * 128:(fcx + 1) * 128]
                            src3 = tp[:, 0:256].rearrange(
                                "p (h f) -> p h f", h=2)
                            if (s + fcx) % 2 == 0:
                                nc.scalar.activation(dst, src3, AF.Copy)
                            else:
                                nc.vector.tensor_scalar(dst, src3, 1.0, 0.0,
                                                        A.mult, A.add)
                    for half in range(2):
                        ps = aps.tile([128, 2, 512], F32, name=f"apst{s}_{half}",
                                      tag="apst")
                        for sub in range(2):
                            fcx = half * 2 + sub
                            nc.tensor.matmul(
                                ps[:, sub, :ROI],
                                hrow[:, slot, 0, fcx * 128:(fcx + 1) * 128],
                                ma, start=True, stop=False)
                            nc.tensor.matmul(
                                ps[:, sub, :ROI],
                                hrow[0:72, slot, 1, fcx * 128:(fcx + 1) * 128],
                                mb[0:72], start=False, stop=True,
                                skip_group_check=True)
                        nc.scalar.activation(
                            bufA[:, half * 2:half * 2 + 2,
                                 s * ROI:(s + 1) * ROI],
                            ps[:, :, :ROI], AF.Copy)

            # ================== main ==================
            for l in [ll % L for ll in range(L * LAYER_REP)]:
                wt = load_weights(l)
                agg(l, first=(l == 0))
                st6a = stp.tile([128, FC, NBLK, 6], F32, name="st6a", tag="st6")
                dense(wt, 0, bufA, bufB, st6a)
                s1, t1 = bn_sync(st6a, NBLK, l, 0, 1, f"a{l}_{id(wt)}")
                apply_elu(bufB, bufC, s1, t1, f"a{l}_{id(wt)}")
                st6b = stp.tile([128, FC, NBLK, 6], F32, name="st6b", tag="st6")
                dense(wt, 1, bufC, bufA, st6b)
                s2, t2 = bn_sync(st6b, NBLK, l, 2, 3, f"b{l}_{id(wt)}")
                apply_elu(bufA, bufB, s2, t2, f"b{l}_{id(wt)}")
                # BN3: stats over w (bufB): sum via DVE ts-accum chunks,
                # sumsq via one ACT Square+accum per fo (garbage into dead bufA)
                swc = stt.tile([128, FC, 7], F32, name=f"swc{l}_{id(wt)}", tag="swc")
                sw = stt.tile([128, FC], F32, name=f"sw{l}_{id(wt)}", tag="sw")
                sw2 = stt.tile([128, FC], F32, name=f"sw2{l}_{id(wt)}", tag="sw2")
                NAB3 = (RPC + 1023) // 1024
                for fo in range(FC):
                    for rb in range(NAB3):
                        off = rb * 1024
                        n = min(1024, RPC - off)
                        dmp = esc.tile([128, 1024], F16,
                                       name=f"dmp{l}_{fo}_{rb}_{id(wt)}", tag="eblk")
                        nc.vector.tensor_scalar(dmp[:, :n], bufB[:, fo, off:off + n],
                                                1.0, 0.0, A.mult, A.add,
                                                accum_out=swc[:, fo, rb:rb + 1])
                    nc.scalar.activation(bufA[:, fo, :RPC], bufB[:, fo, :RPC],
                                         AF.Square, accum_out=sw2[:, fo:fo + 1])
                nc.vector.tensor_reduce(sw[:], swc[:], mybir.AxisListType.X, A.add)
                mn3 = stt.tile([128, FC], F32, name=f"mn3{l}_{id(wt)}", tag="mn3")
                vr3 = stt.tile([128, FC], F32, name=f"vr3{l}_{id(wt)}", tag="vr3")
                nc.vector.tensor_scalar(mn3[:], sw[:], 1.0 / NLOC, 0.0, A.mult, A.add)
                nc.vector.tensor_scalar(vr3[:], sw2[:], 1.0 / NLOC, 0.0, A.mult, A.add)
                tmp3 = stt.tile([128, FC], F32, name=f"tmp3{l}_{id(wt)}", tag="tmp3")
                nc.vector.tensor_tensor(tmp3[:], mn3[:], mn3[:], A.mult)
                nc.vector.tensor_tensor(vr3[:], vr3[:], tmp3[:], A.subtract)
                s3, t3 = bn_sync(None, NBLK, l, 4, 5, f"c{l}_{id(wt)}",
                                 mv=(mn3[:], vr3[:]))
                apply_elu(bufB, bufC, s3, t3, f"c{l}_{id(wt)}")

            # ---- final head ----
            xmT = big.tile([128, FC, S], F32)
            for fcx in range(FC):
                nc.vector.tensor_reduce(
                    xmT[:, fcx, :],
                    bufC[:, fcx, :RPC].rearrange("p (s r) -> p s r", r=ROI),
                    mybir.AxisListType.X, A.add)
            gin = dram.tile([128, FC * S], F32, name="gin")
            gout = dram.tile([NCORES, 128, FC * S], F32, name="gout",
                             addr_space="Shared")
            nc.sync.dma_start(gin[:], xmT[:].rearrange("p c s -> p (c s)"))
            nc.gpsimd.collective_compute(
                "AllGather", A.bypass, ins=[gin[:].opt()], outs=[gout[:].opt()],
                replica_groups=[list(range(NCORES))])
            xa = big.tile([128, FC, NCORES, S], F16)
            nc.gpsimd.dma_start(
                xa[:], gout[:].rearrange("r p (c s) -> p c r s", c=FC))
            # zm.T = Wm1.T @ xa  (fo=256 -> 2 chunks)
            zt = big.tile([128, 2, B], F32)
            st6f = stp.tile([128, 2, 1, 6], F32, name="st6f", tag="st6f")
            for fo in range(2):
                ps = aps.tile([128, B], F32, name=f"fps{fo}", tag="apst")
                for fi in range(FC):
                    nc.tensor.matmul(ps[:], wm1t[:, fi, fo * 128:(fo + 1) * 128],
                                     xa[:, fi], start=(fi == 0),
                                     stop=(fi == FC - 1))
                nc.scalar.activation(zt[:, fo, :], ps[:], AF.Copy)
                nc.vector.bn_stats(st6f[:, fo, 0], zt[:, fo, :])
            # local BN (all 256 rows present) + relu
            agf = stt.tile([128, 2, 2], F32, name="agf")
            for fo in range(2):
                nc.vector.bn_aggr(agf[:, fo], st6f[:, fo, 0])
            vgf = stt.tile([128, 2], F32, name="vgf")
            nc.vector.tensor_copy(vgf[:], agf[:, :, 1:2].rearrange("p c o -> p (c o)"))
            nc.vector.tensor_scalar(vgf[:], vgf[:], 1.0, BN_EPS, A.mult, A.add)
            nc.scalar.activation(vgf[:], vgf[:], AF.Ln, bias=0.0, scale=1.0)
            nc.scalar.activation(vgf[:], vgf[:], AF.Exp, bias=0.0, scale=-0.5)
            sf = stt.tile([128, 2], F32, name="sf")
            tf = stt.tile([128, 2], F32, name="tf")
            nc.vector.tensor_tensor(sf[:], vgf[:], fbnt[:, 0:2], A.mult)
            nc.vector.tensor_tensor(tf[:], agf[:, :, 0:1].rearrange("p c o -> p (c o)"), sf[:], A.mult)
            nc.vector.tensor_tensor(tf[:], fbnt[:, 2:4], tf[:], A.subtract)
            rt = big.tile([128, 2, B], F16)
            for fo in range(2):
                nc.scalar.activation(rt[:, fo, :], zt[:, fo, :], AF.Relu,
                                     bias=tf[:, fo:fo + 1], scale=sf[:, fo:fo + 1])
            psy = aps.tile([128, B], F32, name="psy", tag="apst")
            for fo in range(2):
                nc.tensor.matmul(psy[0:2, :], wm2t[:, fo, :], rt[:, fo, :],
                                 start=(fo == 0), stop=(fo == 1))
            ysb = big.tile([128, B], F32)
            nc.vector.tensor_scalar(ysb[0:2, :], psy[0:2, :], 1.0,
                                    fbnt[0:2, 4:5], A.mult, A.add)
            nc.sync.dma_start(y.ap().rearrange("b t -> t b"), ysb[0:2, :])
    nc.compile()
    return nc


_NC_CACHE = None


def _get_nc():
    global _NC_CACHE
    if _NC_CACHE is None:
        _NC_CACHE = build_nc()
    return _NC_CACHE


def _prep_inputs(x, a, eps, W1, W2, gl_, bl_, g1, be1, g2, be2,
                 gm, betam, Wm1, bm2, Wm2):
    f16 = np.float16
    mask = (np.asarray(a) != 0).astype(np.float32)          # [b, i, j]
    maskT = np.ascontiguousarray(mask.transpose(0, 2, 1))   # [b, j, i]
    eye = np.eye(ROI, dtype=np.float32)
    mk = np.empty((L, B, ROI, ROI), dtype=f16)
    for l in range(L):
        mk[l] = (maskT + float(eps[l]) * eye).astype(f16)
    x_row = np.asarray(x).astype(f16)                        # [b, roi, T]
    w12 = np.empty((L, 2, 128, FC, T), dtype=f16)
    for l in range(L):
        w12[l, 0] = np.asarray(W1[l]).reshape(FC, 128, T).transpose(1, 0, 2)
        w12[l, 1] = np.asarray(W2[l]).reshape(FC, 128, T).transpose(1, 0, 2)
    bnp = np.empty((L, 6, 128, FC), dtype=np.float32)
    for l in range(L):
        for k, p in enumerate((g1[l], be1[l], g2[l], be2[l], gl_[l], bl_[l])):
            bnp[l, k] = np.asarray(p).reshape(FC, 128).T
    wm1p = (np.asarray(Wm1) / ROI).reshape(FC, 128, 256).transpose(1, 0, 2).astype(f16)
    wm2p = np.asarray(Wm2).reshape(2, 128, 2).transpose(1, 0, 2).astype(f16)
    fbn = np.zeros((128, 5), dtype=np.float32)
    fbn[:, 0:2] = np.asarray(gm).reshape(2, 128).T
    fbn[:, 2:4] = np.asarray(betam).reshape(2, 128).T
    fbn[0:2, 4] = np.asarray(bm2)
    return x_row, mk, w12, bnp, wm1p, wm2p, fbn


def make_in_maps(inputs):
    x_row, mk, w12, bnp, wm1p, wm2p, fbn = _prep_inputs(
        inputs['x'], inputs['a'], inputs['eps'], inputs['W1'], inputs['W2'],
        inputs['gl'], inputs['bl'], inputs['g1'], inputs['be1'], inputs['g2'],
        inputs['be2'], inputs['gm'], inputs['betam'], inputs['Wm1'],
        inputs['bm2'], inputs['Wm2'])
    idm = np.eye(128, dtype=np.float16)
    in_maps = []
    for c in range(NCORES):
        sl = slice(c * S, (c + 1) * S)
        in_maps.append({
            "xr": np.ascontiguousarray(x_row[sl]),
            "mk": np.ascontiguousarray(mk[:, sl]),
            "w12": w12, "bnp": bnp, "wm1": wm1p, "wm2": wm2p, "fbn": fbn,
            "idm": idm,
        })
    return in_maps


def kernel(x, a, eps, W1, b1, g1, be1, W2, b2, g2, be2, gl, bl,
           Wm1, bm1, gm, betam, Wm2, bm2):
    in_maps = make_in_maps(dict(x=x, a=a, eps=eps, W1=W1, W2=W2, gl=gl, bl=bl,
                                g1=g1, be1=be1, g2=g2, be2=be2, gm=gm,
                                betam=betam, Wm1=Wm1, bm2=bm2, Wm2=Wm2))
    nc = _get_nc()
    res = run_bass_kernel_spmd(nc, in_maps, core_ids=list(range(NCORES)))
    return res.results[0]["y"].astype(np.float32)


# revision 29
# speedup vs baseline: 2.0369x; 2.0369x over previous
"""Trainium2 Bass kernel for nn_Baseline_GNN (gnn_message_passing).

Data-parallel over batch across 8 NeuronCores. Per-core pipeline (fp16
activations, fp32 PSUM/stats):

  per layer l (3x):
    AGG:   v.T = (maskT + eps*I)_s @ h_s per sample via PE (mask stationary
           chunks are h_row slices, moving operand is maskT), output
           feature-major in PSUM -> SBUF fp16.
    Z1:    z1.T = W1.T-chunks @ v.T   (PE), PSUM->SBUF fp16 copy (ACT) +
           bn_stats (DVE) per tile.
    BN1:   cross-core AllReduce of (sum, sumsq) per feature -> scale/shift.
    ELU:   u = min(exp(n),1)-1+relu(n), n = s*z+t   (ACT exp/relu + DVE).
    Z2/BN2/ELU -> w ; BN3 (stats via bn_stats on w) /ELU -> h'.
    h'.T -> h_row via per-sample DMA transposes (fp16).
  final: xm = row-sum over roi (DVE reduce; 1/200 folded into Wm1),
         AllGather xm, replicated tiny MLP with local BN stats, y (256,2).

b1/b2/bm1 are mathematically dropped (train-mode BN subtracts the mean, so
per-feature constant biases cancel exactly).
"""
import numpy as np
import ml_dtypes

import concourse.bass as bass
import concourse.mybir as mybir
import concourse.tile as tile
import concourse.bacc as bacc
from concourse.bass_utils import run_bass_kernel_spmd

F32 = mybir.dt.float32
F16 = mybir.dt.float16
AF = mybir.ActivationFunctionType
A = mybir.AluOpType

B, ROI, T, L = 256, 200, 512, 3
NCORES = 8
S = B // NCORES            # samples per core
RPC = S * ROI              # rows per core (6400)
FC = T // 128              # feature chunks (4)
NBLK = (RPC + 511) // 512  # 13 dense row-blocks (12x512 + 256)
PADC = RPC + 256           # padded free dim for transpose source buffer
NG = float(B * ROI)        # global BN row count
NLOC = float(RPC)
BN_EPS = 1e-5


def _blk(rb):
    off = rb * 512
    return off, min(512, RPC - off)


import os
LAYER_REP = int(os.environ.get("K_LAYER_REP", "1"))
SKIP_AR = os.environ.get("K_SKIP_AR", "") == "1"
NO_APPLY = os.environ.get("K_NO_APPLY", "") == "1"
NO_STATS = os.environ.get("K_NO_STATS", "") == "1"
NO_TRANS = os.environ.get("K_NO_TRANS", "") == "1"
NO_DENSE = os.environ.get("K_NO_DENSE", "") == "1"
NO_AGG = os.environ.get("K_NO_AGG", "") == "1"


def build_nc():
    nc = bacc.Bacc("TRN2", target_bir_lowering=False, debug=False,
                   num_devices=NCORES)

    xr = nc.dram_tensor("xr", [S, ROI, T], F16, kind="ExternalInput")
    mk = nc.dram_tensor("mk", [L, S, ROI, ROI], F16, kind="ExternalInput")
    w12 = nc.dram_tensor("w12", [L, 2, 128, FC, T], F16, kind="ExternalInput")
    bnp = nc.dram_tensor("bnp", [L, 6, 128, FC], F32, kind="ExternalInput")
    wm1 = nc.dram_tensor("wm1", [128, FC, 256], F16, kind="ExternalInput")
    wm2 = nc.dram_tensor("wm2", [128, 2, 2], F16, kind="ExternalInput")
    fbn = nc.dram_tensor("fbn", [128, 5], F32, kind="ExternalInput")
    idm = nc.dram_tensor("idm", [128, 128], F16, kind="ExternalInput")
    y = nc.dram_tensor("y", [B, 2], F32, kind="ExternalOutput")

    with tile.TileContext(nc) as tc:
        with (
            tc.tile_pool(name="big", bufs=1) as big,
            tc.tile_pool(name="wts", bufs=1) as wts,
            tc.tile_pool(name="mskp", bufs=3) as mskp,
            tc.tile_pool(name="esc", bufs=3) as esc,
            tc.tile_pool(name="stp", bufs=2) as stp,
            tc.tile_pool(name="stt", bufs=4) as stt,
            tc.tile_pool(name="dram", bufs=1, space="DRAM") as dram,
            tc.tile_pool(name="aps", bufs=2, space="PSUM") as aps,
            tc.tile_pool(name="dps", bufs=4, space="PSUM") as dps,
        ):
            # --- persistent big activation buffers ---
            bufA = big.tile([128, FC, RPC], F16)          # vT / z2T
            bufB = big.tile([128, FC, RPC], F16)          # z1T / wT
            bufC = big.tile([128, FC, PADC], F16)         # uT / h'T (padded)
            hrow = big.tile([128, 6, 2, T], F16)          # slots x (a,b) halves
            idt = big.tile([128, 128], F16)
            nc.sync.dma_start(idt[:], idm.ap())

            nc.vector.memset(bufC[:, :, RPC:], 0.0)
            bnpt = big.tile([128, L, 6, FC], F32)
            nc.sync.dma_start(bnpt[:], bnp.ap().rearrange("l k p c -> p l k c"))
            fbnt = big.tile([128, 5], F32)
            nc.sync.dma_start(fbnt[:], fbn.ap())
            wm1t = big.tile([128, FC, 256], F16)
            nc.sync.dma_start(wm1t[:], wm1.ap())
            wm2t = big.tile([128, 2, 2], F16)
            nc.sync.dma_start(wm2t[:], wm2.ap())

            def load_weights(l):
                wt = wts.tile([128, 2, FC, T], F16, name=f"wt{np.random.randint(1<<30)}", tag="wt")
                nc.sync.dma_start(wt[:], w12.ap()[l].rearrange("w p c t -> p w c t"))
                return wt

            def bn_sync(st6, nchunks, l, gk, bek, tag, mv=None):
                """Aggregate bn_stats chunks, AllReduce, return (s,t) (128,FC)."""
                if mv is None:
                    ag = stt.tile([128, FC, 2], F32, name=f"ag{tag}", tag="ag")
                    if NO_STATS:
                        nc.vector.memset(st6[:], 1.0)
                    for fo in range(FC):
                        nc.vector.bn_aggr(
                            ag[:, fo], st6[:, fo, :nchunks].rearrange("p c s -> p (c s)"))
                    mean = ag[:, :, 0:1].rearrange("p c o -> p (c o)")
                    var = ag[:, :, 1:2].rearrange("p c o -> p (c o)")
                else:
                    mean, var = mv
                pay = stt.tile([128, 2 * FC], F32, name=f"pay{tag}", tag="pay")
                msq = stt.tile([128, FC], F32, name=f"msq{tag}", tag="msq")
                nc.vector.tensor_tensor(msq[:], mean, mean, A.mult)
                nc.vector.tensor_tensor(pay[:, FC:], msq[:], var, A.add)
                nc.vector.tensor_scalar(pay[:, FC:], pay[:, FC:], NLOC, 0.0,
                                        A.mult, A.add)
                nc.vector.tensor_scalar(pay[:, :FC], mean, NLOC, 0.0,
                                        A.mult, A.add)
                bin_ = dram.tile([128, 2 * FC], F32, name=f"bin{tag}")
                bout = dram.tile([128, 2 * FC], F32, name=f"bout{tag}",
                                 addr_space="Shared")
                nc.sync.dma_start(bin_[:], pay[:])
                gp = stt.tile([128, 2 * FC], F32, name=f"gp{tag}", tag="gp")
                if SKIP_AR:
                    nc.sync.dma_start(gp[:], bin_[:])
                else:
                    nc.gpsimd.collective_compute(
                        "AllReduce", A.add, ins=[bin_[:].opt()], outs=[bout[:].opt()],
                        replica_groups=[list(range(NCORES))])
                    nc.sync.dma_start(gp[:], bout[:])
                mg = stt.tile([128, FC], F32, name=f"mg{tag}", tag="mg")
                vg = stt.tile([128, FC], F32, name=f"vg{tag}", tag="vg")
                nc.vector.tensor_scalar(mg[:], gp[:, :FC], 1.0 / NG, 0.0,
                                        A.mult, A.add)
                nc.vector.tensor_scalar(vg[:], gp[:, FC:], 1.0 / NG, 0.0,
                                        A.mult, A.add)
                nc.vector.tensor_tensor(msq[:], mg[:], mg[:], A.mult)
                nc.vector.tensor_tensor(vg[:], vg[:], msq[:], A.subtract)
                # rstd = exp(-0.5*ln(var+eps)) ; ln & exp share one ACT table set
                nc.vector.tensor_scalar(vg[:], vg[:], 1.0, BN_EPS, A.mult, A.add)
                nc.scalar.activation(vg[:], vg[:], AF.Ln, bias=0.0, scale=1.0)
                nc.scalar.activation(vg[:], vg[:], AF.Exp, bias=0.0, scale=-0.5)
                st_s = stt.tile([128, FC], F32, name=f"s{tag}", tag="s")
                st_t = stt.tile([128, FC], F32, name=f"t{tag}", tag="t")
                nc.vector.tensor_tensor(st_s[:], vg[:], bnpt[:, l, gk], A.mult)
                nc.vector.tensor_tensor(msq[:], mg[:], st_s[:], A.mult)
                nc.vector.tensor_tensor(st_t[:], bnpt[:, l, bek], msq[:],
                                        A.subtract)
                return st_s, st_t

            def apply_elu(zT, uT, st_s, st_t, tag):
                """u = ELU(n) = max(n, min(exp(n),1)-1), n = s*z+t, blockwise."""
                NAB = (RPC + 1023) // 1024
                if NO_APPLY:
                    for fc in range(FC):
                        nc.vector.tensor_copy(uT[:, fc, :RPC], zT[:, fc, :RPC])
                    return
                for rb in range(NAB):
                    off = rb * 1024
                    n = min(1024, RPC - off)
                    for fc in range(FC):
                        src = zT[:, fc, off:off + n]
                        sA = st_s[:, fc:fc + 1]
                        tA = st_t[:, fc:fc + 1]
                        e = esc.tile([128, 1024], F16, name=f"e{tag}_{rb}_{fc}",
                                     tag="eblk")
                        r = esc.tile([128, 1024], F16, name=f"r{tag}_{rb}_{fc}",
                                     tag="rblk")
                        nc.scalar.activation(e[:, :n], src, AF.Exp,
                                             bias=tA, scale=sA)
                        nc.vector.tensor_scalar(r[:, :n], src, sA, tA,
                                                A.mult, A.add)
                        nc.vector.tensor_scalar(e[:, :n], e[:, :n], 1.0, -1.0,
                                                A.min, A.add)
                        nc.vector.tensor_tensor(uT[:, fc, off:off + n],
                                                r[:, :n], e[:, :n], A.max)

            def dense(wt, wi, srcT, dstT, st6):
                """dstT = (W.T @ srcT) per chunk; PSUM->SBUF + bn_stats."""
                for rb in range(NBLK):
                    off, n = _blk(rb)
                    for fo in range(FC):
                        ps = dps.tile([128, 512], F32,
                                      name=f"dps{rb}_{fo}", tag="dpst")
                        for fi in range(FC) if not NO_DENSE else [0]:
                            nc.tensor.matmul(
                                ps[:, :n],
                                wt[:, wi, fi, fo * 128:(fo + 1) * 128],
                                srcT[:, fi, off:off + n],
                                start=(fi == 0),
                                stop=True if NO_DENSE else (fi == FC - 1))
                        nc.scalar.activation(dstT[:, fo, off:off + n],
                                             ps[:, :n], AF.Copy)
                        if not NO_STATS:
                            nc.vector.bn_stats(st6[:, fo, rb],
                                               dstT[:, fo, off:off + n])

            def agg(l, first):
                """Per-sample aggregation: vT (bufA) = (maskT_eps) @ h."""
                if NO_AGG:
                    nc.vector.memset(bufA[:, :, 0:4], 0.0)
                    return
                mag = mbg = None
                for s in range(S):
                    slot = s % 6
                    mslot = s % 4
                    if s % 4 == 0:
                        mag = mskp.tile([128, 4, ROI], F16,
                                        name=f"ma{l}_{s}_{id(wt)}", tag="ma")
                        mbg = mskp.tile([128, 4, ROI], F16,
                                        name=f"mb{l}_{s}_{id(wt)}", tag="mb")
                        nc.sync.dma_start(
                            mag[:], mk.ap()[l, s:s + 4, 0:128, :].rearrange(
                                "s j i -> j s i"))
                        nc.sync.dma_start(
                            mbg[:72], mk.ap()[l, s:s + 4, 128:200, :].rearrange(
                                "s j i -> j s i"))
                    ma = mag[:, mslot]
                    mb = mbg[:, mslot]
                    if first:
                        nc.sync.dma_start(hrow[:, slot, 0, :], xr.ap()[s, 0:128, :])
                        nc.sync.dma_start(hrow[0:72, slot, 1, :], xr.ap()[s, 128:200, :])
                    elif not NO_TRANS:
                        for fcx in range(FC):
                            c0 = s * ROI
                            tp = dps.tile([128, 512], F16,
                                          name=f"tp{l}_{s}_{fcx}", tag="dpst")
                            nc.tensor.matmul(tp[:, 0:128],
                                             bufC[:, fcx, c0:c0 + 128], idt[:],
                                             is_transpose=True, start=True,
                                             stop=False)
                            nc.tensor.matmul(tp[:, 128:256],
                                             bufC[:, fcx, c0 + 128:c0 + 256],
                                             idt[:], is_transpose=True,
                                             start=False, stop=True,
                                             skip_group_check=True)
                            dst = hrow[:, slot, :, fcx * 128:(fcx + 1) * 128]
                            src3 = tp[:, 0:256].rearrange(
                                "p (h f) -> p h f", h=2)
                            if (s + fcx) % 2 == 0:
                                nc.scalar.activation(dst, src3, AF.Copy)
                            else:
                                nc.vector.tensor_scalar(dst, src3, 1.0, 0.0,
                                                        A.mult, A.add)
                    for half in range(2):
                        ps = aps.tile([128, 2, 512], F32, name=f"apst{s}_{half}",
                                      tag="apst")
                        for sub in range(2):
                            fcx = half * 2 + sub
                            nc.tensor.matmul(
                                ps[:, sub, :ROI],
                                hrow[:, slot, 0, fcx * 128:(fcx + 1) * 128],
                                ma, start=True, stop=False)
                            nc.tensor.matmul(
                                ps[:, sub, :ROI],
                                hrow[0:72, slot, 1, fcx * 128:(fcx + 1) * 128],
                                mb[0:72], start=False, stop=True,
                                skip_group_check=True)
                        nc.scalar.activation(
                            bufA[:, half * 2:half * 2 + 2,
                                 s * ROI:(s + 1) * ROI],
                            ps[:, :, :ROI], AF.Copy)

            # ================== main ==================
            for l in [ll % L for ll in range(L * LAYER_REP)]:
                wt = load_weights(l)
                agg(l, first=(l == 0))
                st6a = stp.tile([128, FC, NBLK, 6], F32, name="st6a", tag="st6")
                dense(wt, 0, bufA, bufB, st6a)
                s1, t1 = bn_sync(st6a, NBLK, l, 0, 1, f"a{l}_{id(wt)}")
                apply_elu(bufB, bufC, s1, t1, f"a{l}_{id(wt)}")
                st6b = stp.tile([128, FC, NBLK, 6], F32, name="st6b", tag="st6")
                dense(wt, 1, bufC, bufA, st6b)
                s2, t2 = bn_sync(st6b, NBLK, l, 2, 3, f"b{l}_{id(wt)}")
                apply_elu(bufA, bufB, s2, t2, f"b{l}_{id(wt)}")
                # BN3: stats over w (bufB): sum via DVE ts-accum chunks,
                # sumsq via one ACT Square+accum per fo (garbage into dead bufA)
                swc = stt.tile([128, FC, 7], F32, name=f"swc{l}_{id(wt)}", tag="swc")
                sw = stt.tile([128, FC], F32, name=f"sw{l}_{id(wt)}", tag="sw")
                sw2 = stt.tile([128, FC], F32, name=f"sw2{l}_{id(wt)}", tag="sw2")
                NAB3 = (RPC + 1023) // 1024
                for fo in range(FC):
                    for rb in range(NAB3):
                        off = rb * 1024
                        n = min(1024, RPC - off)
                        dmp = esc.tile([128, 1024], F16,
                                       name=f"dmp{l}_{fo}_{rb}_{id(wt)}", tag="eblk")
                        nc.vector.tensor_scalar(dmp[:, :n], bufB[:, fo, off:off + n],
                                                1.0, 0.0, A.mult, A.add,
                                                accum_out=swc[:, fo, rb:rb + 1])
                    nc.scalar.activation(bufA[:, fo, :RPC], bufB[:, fo, :RPC],
                                         AF.Square, accum_out=sw2[:, fo:fo + 1])
                nc.vector.tensor_reduce(sw[:], swc[:], mybir.AxisListType.X, A.add)
                mn3 = stt.tile([128, FC], F32, name=f"mn3{l}_{id(wt)}", tag="mn3")
                vr3 = stt.tile([128, FC], F32, name=f"vr3{l}_{id(wt)}", tag="vr3")
                nc.vector.tensor_scalar(mn3[:], sw[:], 1.0 / NLOC, 0.0, A.mult, A.add)
                nc.vector.tensor_scalar(vr3[:], sw2[:], 1.0 / NLOC, 0.0, A.mult, A.add)
                tmp3 = stt.tile([128, FC], F32, name=f"tmp3{l}_{id(wt)}", tag="tmp3")
                nc.vector.tensor_tensor(tmp3[:], mn3[:], mn3[:], A.mult)
                nc.vector.tensor_tensor(vr3[:], vr3[:], tmp3[:], A.subtract)
                s3, t3 = bn_sync(None, NBLK, l, 4, 5, f"c{l}_{id(wt)}",
                                 mv=(mn3[:], vr3[:]))
                apply_elu(bufB, bufC, s3, t3, f"c{l}_{id(wt)}")

            # ---- final head ----
            xmT = big.tile([128, FC, S], F32)
            for fcx in range(FC):
                nc.vector.tensor_reduce(
                    xmT[:, fcx, :],
                    bufC[:, fcx, :RPC].rearrange("p (s r) -> p s r", r=ROI),
                    mybir.AxisListType.X, A.add)
            gin = dram.tile([128, FC * S], F32, name="gin")
            gout = dram.tile([NCORES, 128, FC * S], F32, name="gout",
                             addr_space="Shared")
            nc.sync.dma_start(gin[:], xmT[:].rearrange("p c s -> p (c s)"))
            nc.gpsimd.collective_compute(
                "AllGather", A.bypass, ins=[gin[:].opt()], outs=[gout[:].opt()],
                replica_groups=[list(range(NCORES))])
            xa = big.tile([128, FC, NCORES, S], F16)
            nc.gpsimd.dma_start(
                xa[:], gout[:].rearrange("r p (c s) -> p c r s", c=FC))
            # zm.T = Wm1.T @ xa  (fo=256 -> 2 chunks)
            zt = big.tile([128, 2, B], F32)
            st6f = stp.tile([128, 2, 1, 6], F32, name="st6f", tag="st6f")
            for fo in range(2):
                ps = aps.tile([128, B], F32, name=f"fps{fo}", tag="apst")
                for fi in range(FC):
                    nc.tensor.matmul(ps[:], wm1t[:, fi, fo * 128:(fo + 1) * 128],
                                     xa[:, fi], start=(fi == 0),
                                     stop=(fi == FC - 1))
                nc.scalar.activation(zt[:, fo, :], ps[:], AF.Copy)
                nc.vector.bn_stats(st6f[:, fo, 0], zt[:, fo, :])
            # local BN (all 256 rows present) + relu
            agf = stt.tile([128, 2, 2], F32, name="agf")
            for fo in range(2):
                nc.vector.bn_aggr(agf[:, fo], st6f[:, fo, 0])
            vgf = stt.tile([128, 2], F32, name="vgf")
            nc.vector.tensor_copy(vgf[:], agf[:, :, 1:2].rearrange("p c o -> p (c o)"))
            nc.vector.tensor_scalar(vgf[:], vgf[:], 1.0, BN_EPS, A.mult, A.add)
            nc.scalar.activation(vgf[:], vgf[:], AF.Ln, bias=0.0, scale=1.0)
            nc.scalar.activation(vgf[:], vgf[:], AF.Exp, bias=0.0, scale=-0.5)
            sf = stt.tile([128, 2], F32, name="sf")
            tf = stt.tile([128, 2], F32, name="tf")
            nc.vector.tensor_tensor(sf[:], vgf[:], fbnt[:, 0:2], A.mult)
            nc.vector.tensor_tensor(tf[:], agf[:, :, 0:1].rearrange("p c o -> p (c o)"), sf[:], A.mult)
            nc.vector.tensor_tensor(tf[:], fbnt[:, 2:4], tf[:], A.subtract)
            rt = big.tile([128, 2, B], F16)
            for fo in range(2):
                nc.scalar.activation(rt[:, fo, :], zt[:, fo, :], AF.Relu,
                                     bias=tf[:, fo:fo + 1], scale=sf[:, fo:fo + 1])
            psy = aps.tile([128, B], F32, name="psy", tag="apst")
            for fo in range(2):
                nc.tensor.matmul(psy[0:2, :], wm2t[:, fo, :], rt[:, fo, :],
                                 start=(fo == 0), stop=(fo == 1))
            ysb = big.tile([128, B], F32)
            nc.vector.tensor_scalar(ysb[0:2, :], psy[0:2, :], 1.0,
                                    fbnt[0:2, 4:5], A.mult, A.add)
            nc.sync.dma_start(y.ap().rearrange("b t -> t b"), ysb[0:2, :])
    nc.compile()
    return nc


_NC_CACHE = None


def _get_nc():
    global _NC_CACHE
    if _NC_CACHE is None:
        _NC_CACHE = build_nc()
    return _NC_CACHE


def _prep_inputs(x, a, eps, W1, W2, gl_, bl_, g1, be1, g2, be2,
                 gm, betam, Wm1, bm2, Wm2):
    f16 = np.float16
    mask = (np.asarray(a) != 0).astype(np.float32)          # [b, i, j]
    maskT = np.ascontiguousarray(mask.transpose(0, 2, 1))   # [b, j, i]
    eye = np.eye(ROI, dtype=np.float32)
    mk = np.empty((L, B, ROI, ROI), dtype=f16)
    for l in range(L):
        mk[l] = (maskT + float(eps[l]) * eye).astype(f16)
    x_row = np.asarray(x).astype(f16)                        # [b, roi, T]
    w12 = np.empty((L, 2, 128, FC, T), dtype=f16)
    for l in range(L):
        w12[l, 0] = np.asarray(W1[l]).reshape(FC, 128, T).transpose(1, 0, 2)
        w12[l, 1] = np.asarray(W2[l]).reshape(FC, 128, T).transpose(1, 0, 2)
    bnp = np.empty((L, 6, 128, FC), dtype=np.float32)
    for l in range(L):
        for k, p in enumerate((g1[l], be1[l], g2[l], be2[l], gl_[l], bl_[l])):
            bnp[l, k] = np.asarray(p).reshape(FC, 128).T
    wm1p = (np.asarray(Wm1) / ROI).reshape(FC, 128, 256).transpose(1, 0, 2).astype(f16)
    wm2p = np.asarray(Wm2).reshape(2, 128, 2).transpose(1, 0, 2).astype(f16)
    fbn = np.zeros((128, 5), dtype=np.float32)
    fbn[:, 0:2] = np.asarray(gm).reshape(2, 128).T
    fbn[:, 2:4] = np.asarray(betam).reshape(2, 128).T
    fbn[0:2, 4] = np.asarray(bm2)
    return x_row, mk, w12, bnp, wm1p, wm2p, fbn


def make_in_maps(inputs):
    x_row, mk, w12, bnp, wm1p, wm2p, fbn = _prep_inputs(
        inputs['x'], inputs['a'], inputs['eps'], inputs['W1'], inputs['W2'],
        inputs['gl'], inputs['bl'], inputs['g1'], inputs['be1'], inputs['g2'],
        inputs['be2'], inputs['gm'], inputs['betam'], inputs['Wm1'],
        inputs['bm2'], inputs['Wm2'])
    idm = np.eye(128, dtype=np.float16)
    in_maps = []
    for c in range(NCORES):
        sl = slice(c * S, (c + 1) * S)
        in_maps.append({
            "xr": np.ascontiguousarray(x_row[sl]),
            "mk": np.ascontiguousarray(mk[:, sl]),
            "w12": w12, "bnp": bnp, "wm1": wm1p, "wm2": wm2p, "fbn": fbn,
            "idm": idm,
        })
    return in_maps


def kernel(x, a, eps, W1, b1, g1, be1, W2, b2, g2, be2, gl, bl,
           Wm1, bm1, gm, betam, Wm2, bm2):
    in_maps = make_in_maps(dict(x=x, a=a, eps=eps, W1=W1, W2=W2, gl=gl, bl=bl,
                                g1=g1, be1=be1, g2=g2, be2=be2, gm=gm,
                                betam=betam, Wm1=Wm1, bm2=bm2, Wm2=Wm2))
    nc = _get_nc()
    res = run_bass_kernel_spmd(nc, in_maps, core_ids=list(range(NCORES)))
    return res.results[0]["y"].astype(np.float32)
